# revision 10
# baseline (speedup 1.0000x reference)
"""Trainium2 Bass kernel for nn_BaseRecommender (masked top-k recommendation).

Strategy (hardcoded, self-contained):
  - Shard the item embedding table column-wise (item dim) across 8 cores:
    12500 items/core, zero-padded to 13312 = 13 matmul chunks x 1024.
  - Replicate u_e = all_embed[user_list] (gathered + transposed on host).
  - Per core, per 128-row tile: f32r matmuls -> PSUM [128,1024] fp32 chunks.
    The expensive part of top-k on TRN2 is the DVE max8/max_index scan
    (1 elem/cycle, no fast modes), so instead of scanning all 13312 columns
    twice we first collapse them with a bf16 max-folding tree:
      * ACT_CHUNKS chunks: scalar-engine copy PSUM->SBUF with bf16 downcast,
        then one merged DVE tensor_max folds the region in half (bf16
        tensor_tensor runs in 2x_1p mode = 2 elem/cycle);
      * the remaining chunks: DVE tensor_max folds each PSUM chunk directly
        into bf16 SBUF (fp32 1x mode, but skips the Act copy), balancing the
        Act and DVE engines.
    Three more bf16 fold levels shrink the row to FINAL_W=832 group maxima
    (each group = 16 original columns), then max8 + max_index scan only 832
    columns.  DVE work drops ~3x vs scanning raw scores.
  - Host: decodes the top-8 groups per (core,row) via a static group table,
    recomputes exact fp32 scores for all 16 members of each group, scores
    the maskable region (global item cols [0,1024), the only range the
    reference ever masks) exactly, merges and re-selects the global top-k.
    A guard recomputes any core-shard whose 8th group max could still reach
    the row's top-20 (covers bf16 rounding + f32r noise + any top-8
    truncation), so the result is exact.
"""

import os
import sys

import numpy as np

try:
    import concourse  # noqa: F401
except ImportError:
    for _p in ("/opt/trn_rl_repo", os.path.expanduser("~/.axon_site/_ro/trn_rl_repo")):
        if os.path.isdir(_p):
            sys.path.insert(0, _p)
            try:
                import concourse  # noqa: F401

                break
            except ImportError:
                sys.path.remove(_p)

N_USERS = 100000
N_ITEMS = 100000
EMB = 64
BATCH = 1024
K = 20
NEG = -100000.0
NCORES = 8
ISHARD = N_ITEMS // NCORES  # 12500 items per core
PCH = 1024  # matmul/psum chunk (columns)
NPCH = 13  # psum chunks per core
IPAD = NPCH * PCH  # 13312
ABSORBS = int(os.environ.get("KERN_ABSORBS", "3"))  # PSUM-folded chunks
ACT_CHUNKS = NPCH - ABSORBS  # chunks copied PSUM->SBUF (bf16) by scalar engine
NA = ACT_CHUNKS * PCH  # width of the Act-copied bf16 staging region
REM = ACT_CHUNKS - ABSORBS  # act chunks not used as absorb partners
REMW = REM * PCH  # width of the fold-in-half region
FOLDS = int(os.environ.get("KERN_FOLDS", "4"))  # fold levels; group = 2**FOLDS
GW = 1 << FOLDS  # 16 columns per group
FINAL_W = IPAD >> FOLDS  # 832 group maxima scanned by max8
NSLOT = 8  # candidate groups returned per (core, row)
ROWT = 128
NROWT = BATCH // ROWT  # 8 row tiles
HOST_COLS = 1024  # item columns [0, HOST_COLS) are scored on host (mask range)
REL_EPS = 6e-3  # relative guard margin (bf16 round + f32r matmul noise)
ABS_EPS = 1e-2  # absolute guard margin

_compiled = None


def _build_group_table():
    """Static map: final fold position p -> the GW original local item columns
    it covers (-1 for zero-padding columns >= ISHARD).

    Chunk c covers local item cols [c*PCH, (c+1)*PCH).  Level 1 (f1, width
    IPAD/2): absorb d pairs chunk ACT_CHUNKS+d (PSUM) with partner chunk d
    elementwise -> f1[d*PCH : (d+1)*PCH]; the remaining act chunks
    ABSORBS..ACT_CHUNKS-1 are folded in half -> f1[ABSORBS*PCH :].  Levels
    2..FOLDS pair p with p + width/2 of the running array.
    """
    half = IPAD // 2
    sets = [None] * half
    for d in range(ABSORBS):
        for t in range(PCH):
            sets[d * PCH + t] = [(ACT_CHUNKS + d) * PCH + t, d * PCH + t]
    base = ABSORBS * PCH
    for j in range(REMW // 2):
        sets[base + j] = [base + j, base + REMW // 2 + j]
    w = half
    for _ in range(FOLDS - 1):
        w //= 2
        sets = [sets[p] + sets[p + w] for p in range(w)]
    assert w == FINAL_W
    tbl = np.full((FINAL_W, GW), -1, dtype=np.int64)
    for p, items in enumerate(sets):
        for j, v in enumerate(items):
            tbl[p, j] = v if v < ISHARD else -1
    return tbl


_GTBL = _build_group_table()


def _build_bass(loop_n=1):
    """Build the per-core Bass program. loop_n > 1 repeats the compute loop
    (hardware For_i) for differential HW timing; loads happen once."""
    from concourse import bacc
    import concourse.mybir as mybir
    from concourse.tile import TileContext

    F32 = mybir.dt.float32
    F32R = mybir.dt.float32r
    BF16 = mybir.dt.bfloat16
    U32 = mybir.dt.uint32

    nc = bacc.Bacc("TRN2", target_bir_lowering=False, debug=False, num_devices=NCORES)
    u_t = nc.dram_tensor("u_t", [EMB, BATCH], F32R, kind="ExternalInput")
    i_t = nc.dram_tensor("i_t", [EMB, IPAD], F32R, kind="ExternalInput")
    cv = nc.dram_tensor("cv", [BATCH, NSLOT], BF16, kind="ExternalOutput")
    ci = nc.dram_tensor("ci", [BATCH, NSLOT], U32, kind="ExternalOutput")

    with TileContext(nc) as tc:
        with (
            tc.tile_pool(name="consts", bufs=1) as consts,
            tc.tile_pool(name="psum", bufs=4, space="PSUM") as psum,
            tc.tile_pool(name="work", bufs=2) as work,
            tc.tile_pool(name="cand", bufs=2) as cand,
        ):
            u_sb = consts.tile([EMB, BATCH], F32R, tag="u_sb")
            nc.sync.dma_start(u_sb[:], u_t[:])
            i_sb = []
            for c in range(NPCH):
                t = consts.tile([EMB, PCH], F32R, tag=f"i_sb{c}")
                nc.sync.dma_start(t[:], i_t[:, c * PCH : (c + 1) * PCH])
                i_sb.append(t)

            def body():
                for rt in range(NROWT):
                    s_bf = work.tile([ROWT, NA], BF16, tag="s_bf")
                    f1 = work.tile([ROWT, IPAD // 2], BF16, tag="f1")
                    fl = [
                        work.tile(
                            [ROWT, IPAD >> (l + 1)],
                            BF16,
                            name=f"f{l + 1}",
                            tag=f"f{l + 1}",
                        )
                        for l in range(1, FOLDS)
                    ]
                    cv_t = cand.tile([ROWT, NSLOT], BF16, tag="cv_t")
                    ci_t = cand.tile([ROWT, NSLOT], U32, tag="ci_t")
                    lhs = u_sb[:, rt * ROWT : (rt + 1) * ROWT]
                    with nc.allow_low_precision(reason="bf16 candidate scores"):
                        for c in range(NPCH):
                            ps = psum.tile([ROWT, PCH], F32, tag="ps")
                            nc.tensor.matmul(
                                ps[:, 0:512], lhs, i_sb[c][:, 0:512], start=True, stop=True
                            )
                            nc.tensor.matmul(
                                ps[:, 512:1024],
                                lhs,
                                i_sb[c][:, 512:1024],
                                start=True,
                                stop=True,
                            )
                            if c < ACT_CHUNKS:
                                nc.scalar.copy(s_bf[:, c * PCH : (c + 1) * PCH], ps[:])
                            else:
                                d = c - ACT_CHUNKS
                                nc.vector.tensor_max(
                                    f1[:, d * PCH : (d + 1) * PCH],
                                    ps[:],
                                    s_bf[:, d * PCH : (d + 1) * PCH],
                                )
                        nc.vector.tensor_max(
                            f1[:, ABSORBS * PCH : IPAD // 2],
                            s_bf[:, ABSORBS * PCH : ABSORBS * PCH + REMW // 2],
                            s_bf[:, ABSORBS * PCH + REMW // 2 : NA],
                        )
                        prev = f1
                        for l in range(1, FOLDS):
                            w = IPAD >> (l + 1)
                            nc.vector.tensor_max(
                                fl[l - 1][:], prev[:, 0:w], prev[:, w : 2 * w]
                            )
                            prev = fl[l - 1]
                        nc.vector.max(cv_t[:], prev[:])
                        nc.vector.max_index(ci_t[:], cv_t[:], prev[:])
                    nc.sync.dma_start(cv[rt * ROWT : (rt + 1) * ROWT, :], cv_t[:])
                    nc.sync.dma_start(ci[rt * ROWT : (rt + 1) * ROWT, :], ci_t[:])

            if loop_n == 1:
                body()
            else:
                with tc.For_i(0, loop_n, 1):
                    body()

    nc.compile()
    return nc


def _get_compiled():
    global _compiled
    if _compiled is None:
        _compiled = _build_bass()
    return _compiled


def run_device(u_t, i_t_shards, trace=False, **kwargs):
    from concourse.bass_utils import run_bass_kernel_spmd

    nc = _get_compiled()
    in_maps = [{"u_t": u_t, "i_t": i_t_shards[s]} for s in range(NCORES)]
    return run_bass_kernel_spmd(nc, in_maps, list(range(NCORES)), trace=trace, **kwargs)


def make_device_inputs(all_embed, user_list):
    all_embed = np.asarray(all_embed, dtype=np.float32)
    user_list = np.asarray(user_list)
    u_e = all_embed[user_list.astype(np.int64)]  # [BATCH, EMB]
    i_e = all_embed[N_USERS:]  # [I, E]
    u_t = np.ascontiguousarray(u_e.T)  # [EMB, BATCH]
    i_t_shards = []
    for s in range(NCORES):
        sh = np.zeros((EMB, IPAD), dtype=np.float32)
        sh[:, :ISHARD] = i_e[s * ISHARD : (s + 1) * ISHARD].T
        i_t_shards.append(sh)
    return u_e, i_e, u_t, i_t_shards


def _mask_host_scores(s0, pos_pad):
    """Reference masking semantics on the host-scored region: only valid
    positives with local item index < BATCH (== HOST_COLS) are masked."""
    pos_pad = np.asarray(pos_pad)
    item_idx = pos_pad.astype(np.int64) - N_USERS
    valid = (pos_pad >= 0) & (item_idx < HOST_COLS)
    r, c = np.nonzero(valid)
    np.minimum.at(s0, (r, item_idx[r, c]), np.float32(NEG))
    return s0


def postprocess(results, u_e, i_e, pos_pad):
    """Decode candidate groups, recompute exact scores, merge, re-select."""
    slot_v = np.empty((NCORES, BATCH, NSLOT), dtype=np.float32)
    slot_p = np.empty((NCORES, BATCH, NSLOT), dtype=np.int64)
    for s in range(NCORES):
        slot_v[s] = np.asarray(results[s]["cv"], dtype=np.float32)
        slot_p[s] = results[s]["ci"].astype(np.int64)

    # Guard against malformed positions (duplicates handled below).
    slot_ok = (slot_p >= 0) & (slot_p < FINAL_W)
    safe_p = np.where(slot_ok, slot_p, 0)

    # Expand groups -> candidate local item columns [NCORES, BATCH, NSLOT*GW]
    members = _GTBL[safe_p]  # [NCORES, BATCH, NSLOT, GW]
    cand_l = members.reshape(NCORES, BATCH, NSLOT * GW)
    core_base = (np.arange(NCORES, dtype=np.int64) * ISHARD)[:, None, None]
    cand_g = np.where(cand_l >= 0, cand_l + core_base, -1)
    cand_ok = (
        (cand_l >= 0) & slot_ok.repeat(GW, axis=2) & (cand_g >= HOST_COLS)
    )

    # [BATCH, NCORES * NSLOT * GW]
    cand_g = cand_g.transpose(1, 0, 2).reshape(BATCH, -1)
    cand_ok = cand_ok.transpose(1, 0, 2).reshape(BATCH, -1)
    safe_g = np.where(cand_ok, cand_g, 0)
    cand_v = np.einsum("re,rce->rc", u_e, i_e[safe_g], optimize=True).astype(np.float32)
    cand_v[~cand_ok] = -np.inf
    cand_g = np.where(cand_ok, cand_g, -1)

    # Host-exact scores for the maskable region (global item cols [0, 1024)).
    s0 = u_e @ i_e[:HOST_COLS].T  # [BATCH, HOST_COLS] float32
    s0 = _mask_host_scores(s0, pos_pad)
    hp = np.argpartition(-s0, K, axis=1)[:, :K]
    hv = np.take_along_axis(s0, hp, axis=1).astype(np.float32)

    all_v = np.concatenate([hv, cand_v], axis=1)
    all_g = np.concatenate([hp.astype(np.int64), cand_g], axis=1)

    order = np.argsort(-all_v, axis=1, kind="stable")[:, : K + 1]
    rows = np.arange(BATCH)[:, None]
    sel_v = all_v[rows, order]
    v20 = sel_v[:, K - 1]

    # Guard: core's 8th group max (bf16) + margin can still reach the row's
    # 20th exact value -> that core may hide candidates; recompute it exactly.
    # Also trigger on duplicate/oob slot positions (tie-loss safety) when the
    # offending value could matter.
    scale = np.maximum(np.abs(sel_v[:, 0]), 1.0)  # [BATCH]
    margin = REL_EPS * scale + ABS_EPS
    trig = slot_v[:, :, NSLOT - 1] + margin[None, :] >= v20[None, :]  # [NCORES,BATCH]
    for s in range(NCORES):
        dup = np.zeros(BATCH, dtype=bool)
        sp = np.sort(slot_p[s], axis=1)
        dup |= (sp[:, 1:] == sp[:, :-1]).any(axis=1)
        dup |= ~slot_ok[s].all(axis=1)
        if dup.any():
            top_margin = slot_v[s, :, 0] + margin >= v20
            trig[s] |= dup & top_margin
    tie = sel_v[:, K - 1] == sel_v[:, K]
    careful = set(np.nonzero(trig.any(axis=0) | tie)[0].tolist())

    out_idx = np.empty((BATCH, K), dtype=np.int64)
    out_val = np.empty((BATCH, K), dtype=np.float32)

    top_g = all_g[rows, order[:, :K]]
    top_v = sel_v[:, :K]
    for r in range(BATCH):
        o = np.lexsort((top_g[r], -top_v[r]))
        out_idx[r] = top_g[r][o]
        out_val[r] = top_v[r][o]

    for r in careful:
        vals = list(all_v[r].astype(np.float64))
        idxs = list(all_g[r])
        recomputed = set()
        while True:
            vv = np.asarray(vals, dtype=np.float64)
            gg = np.asarray(idxs, dtype=np.int64)
            o = np.lexsort((gg, -vv))[:K]
            tg, tv = gg[o], vv[o]
            r20 = tv[-1]
            trig_r = []
            for s in range(NCORES):
                if s in recomputed:
                    continue
                hit = slot_v[s, r, NSLOT - 1] + margin[r] >= r20
                sp = np.sort(slot_p[s, r])
                if (sp[1:] == sp[:-1]).any() or not slot_ok[s, r].all():
                    hit = hit or (slot_v[s, r, 0] + margin[r] >= r20)
                if hit:
                    trig_r.append(s)
            if not trig_r:
                break
            for s in trig_r:
                recomputed.add(s)
                # invalidate the core's original candidates (superseded by
                # the full-shard recompute; avoids duplicate indices)
                base = K + s * NSLOT * GW
                for j in range(base, base + NSLOT * GW):
                    vals[j] = -np.inf
                    idxs[j] = -1
                lo = max(s * ISHARD, HOST_COLS)
                hi = (s + 1) * ISHARD
                if lo >= hi:
                    continue
                sc = (i_e[lo:hi] @ u_e[r]).astype(np.float32)
                vals.extend(sc.tolist())
                idxs.extend(range(lo, hi))
        out_idx[r] = tg
        out_val[r] = tv.astype(np.float32)

    return out_idx.astype(np.int32) + N_USERS, out_val


def kernel(all_embed, pos_pad, user_list, k):
    pos_pad = np.asarray(pos_pad)
    k = int(k)
    assert k == K, f"kernel hardcoded for k={K}, got {k}"
    u_e, i_e, u_t, i_t_shards = make_device_inputs(all_embed, user_list)
    res = run_device(u_t, i_t_shards)
    return postprocess(res.results, u_e, i_e, pos_pad)


# revision 42
# speedup vs baseline: 2.9064x; 2.9064x over previous
"""Trainium2 Bass kernel for nn_BaseRecommender (masked top-k recommendation).

Strategy (hardcoded, self-contained):
  - Shard the item embedding table column-wise (item dim) across 8 cores:
    12500 items/core, zero-padded to 13312 = 13 matmul chunks x 1024.
  - Replicate u_e = all_embed[user_list] (gathered + transposed on host,
    cast to bf16).
  - PE: bf16 matmuls with 2-way row tiling.  The contraction dim is only 64,
    so u and the item chunks are duplicated into both SBUF partition halves
    and each 1024-col chunk is computed as two concurrent 512-col matmuls on
    PE tiles (0,0) and (64,0).  This (plus bf16 FWL weight loads; f32r
    streams at half rate) took the bare matmul stream from 104us to 27us.
  - Candidate generation: the expensive part of top-k on TRN2 is any full
    DVE scan (max8/max_index are 1 elem/cycle, no fast modes), so the score
    rows are collapsed by a bf16 max-folding tree instead:
      * ACT_CHUNKS chunks: scalar-engine copy PSUM->SBUF with bf16 downcast
        (BIGCOPY pairs two chunks per 2048-wide copy to amortize the ~350
        cycle per-instruction overhead), then one merged DVE tensor_max
        folds the region in half (bf16 tensor_tensor runs 2x_1p = 2
        elem/cycle);
      * ABSORBS chunks: DVE tensor_max folds each PSUM chunk directly
        against an already-copied bf16 partner chunk (one PSUM operand is
        the hardware limit), skipping the Act copy entirely — this balances
        the Act and DVE engines, the joint bottleneck.
    FOLDS-1 more bf16 fold levels shrink each row to FINAL_W group maxima
    (each group = 2**FOLDS original columns), which are DMAed to the host —
    no on-device max8/max_index at all.
  - Host: sorts the 8*FINAL_W group maxima per row, expands the top T
    groups via a static group table, rescores all their members exactly in
    fp32, and grows T until the next unexpanded group max + margin cannot
    reach the row's 20th exact value (margin covers bf16 rounding + bf16
    matmul noise; group maxima are upper bounds up to that margin, so the
    cutoff is sound).  The maskable region (global item cols [0,1024), the
    only range the reference ever masks) is scored exactly on host.  Final
    merge reproduces the reference's value-then-index tie order, so the
    result is exact.
"""

import os
import sys

import numpy as np

try:
    import concourse  # noqa: F401
except ImportError:
    for _p in ("/opt/trn_rl_repo", os.path.expanduser("~/.axon_site/_ro/trn_rl_repo")):
        if os.path.isdir(_p):
            sys.path.insert(0, _p)
            try:
                import concourse  # noqa: F401

                break
            except ImportError:
                sys.path.remove(_p)

N_USERS = 100000
N_ITEMS = 100000
EMB = 64
BATCH = 1024
K = 20
NEG = -100000.0
NCORES = 8
ISHARD = N_ITEMS // NCORES  # 12500 items per core
PCH = 1024  # matmul/psum chunk (columns)
NPCH = 13  # psum chunks per core
IPAD = NPCH * PCH  # 13312
ABSORBS = int(os.environ.get("KERN_ABSORBS", "4"))  # PSUM-folded chunks
DMACH = int(os.environ.get("KERN_DMACH", "0"))  # chunks DMA-copied PSUM->SBUF
assert DMACH == 0, "DMA PSUM->SBUF is rejected by hardware; DMACH retired"
ACT_CHUNKS = NPCH - ABSORBS - DMACH  # chunks copied by the scalar engine
NA = ACT_CHUNKS * PCH  # width of the Act-copied bf16 staging region
REM = ACT_CHUNKS - ABSORBS  # act chunks not used as absorb partners
REMW = REM * PCH  # width of the fold-in-half region
FOLDS = int(os.environ.get("KERN_FOLDS", "5"))  # fold levels; group = 2**FOLDS
HOST_TOPK = int(os.environ.get("KERN_HOSTTOPK", "1"))  # 1: DMA group maxima out
DIAG = int(os.environ.get("KERN_DIAG", "0"))  # timing-only partial pipelines
MM_DT = os.environ.get("KERN_MMDT", "bf16")  # matmul input dtype: f32r | bf16
ROWTILE = int(os.environ.get("KERN_ROWTILE", "1"))  # 1: 2-way PE row tiling (K=64)
GPFOLD = int(os.environ.get("KERN_GPFOLD", "0"))  # 1: fold level 2 on GPSIMD
BIGCOPY = int(os.environ.get("KERN_BIGCOPY", "0"))  # 1: 2048-wide psum/Act copies
WBUFS = int(os.environ.get("KERN_WBUFS", "2"))  # work-pool buffering depth
GW = 1 << FOLDS  # 16 columns per group
FINAL_W = IPAD >> FOLDS  # 832 group maxima scanned by max8
NSLOT = 8  # candidate groups returned per (core, row)
ROWT = 128
NROWT = BATCH // ROWT  # 8 row tiles
HOST_COLS = 1024  # item columns [0, HOST_COLS) are scored on host (mask range)
REL_EPS = 6e-3  # relative guard margin (bf16 round + f32r matmul noise)
ABS_EPS = 1e-2  # absolute guard margin

_compiled = None


def _build_group_table():
    """Static map: final fold position p -> the GW original local item columns
    it covers (-1 for zero-padding columns >= ISHARD).

    Chunk c covers local item cols [c*PCH, (c+1)*PCH).  Level 1 (f1, width
    IPAD/2): absorb d pairs chunk ACT_CHUNKS+d (PSUM) with partner chunk d
    elementwise -> f1[d*PCH : (d+1)*PCH]; the remaining act chunks
    ABSORBS..ACT_CHUNKS-1 are folded in half -> f1[ABSORBS*PCH :].  Levels
    2..FOLDS pair p with p + width/2 of the running array.
    """
    half = IPAD // 2
    sets = [None] * half
    for d in range(ABSORBS):
        for t in range(PCH):
            sets[d * PCH + t] = [(ACT_CHUNKS + d) * PCH + t, d * PCH + t]
    base = ABSORBS * PCH
    for j in range(REMW // 2):
        sets[base + j] = [base + j, base + REMW // 2 + j]
    base2 = base + REMW // 2
    for m in range(DMACH):
        cb = (ACT_CHUNKS + ABSORBS + m) * PCH
        for t in range(PCH // 2):
            sets[base2 + m * (PCH // 2) + t] = [cb + t, cb + PCH // 2 + t]
    w = half
    for _ in range(FOLDS - 1):
        w //= 2
        sets = [sets[p] + sets[p + w] for p in range(w)]
    assert w == FINAL_W
    tbl = np.full((FINAL_W, GW), -1, dtype=np.int64)
    for p, items in enumerate(sets):
        for j, v in enumerate(items):
            tbl[p, j] = v if v < ISHARD else -1
    return tbl


_GTBL = _build_group_table()


def _build_bass(loop_n=1):
    """Build the per-core Bass program. loop_n > 1 repeats the compute loop
    (hardware For_i) for differential HW timing; loads happen once."""
    from concourse import bacc
    import concourse.mybir as mybir
    from concourse.tile import TileContext

    F32 = mybir.dt.float32
    F32R = mybir.dt.float32r
    BF16 = mybir.dt.bfloat16
    U32 = mybir.dt.uint32
    MDT = BF16 if MM_DT == "bf16" else F32R

    nc = bacc.Bacc("TRN2", target_bir_lowering=False, debug=False, num_devices=NCORES)
    u_t = nc.dram_tensor("u_t", [EMB, BATCH], MDT, kind="ExternalInput")
    i_t = nc.dram_tensor("i_t", [EMB, IPAD], MDT, kind="ExternalInput")
    if HOST_TOPK:
        fo = nc.dram_tensor("fo", [BATCH, FINAL_W], BF16, kind="ExternalOutput")
    else:
        cv = nc.dram_tensor("cv", [BATCH, NSLOT], BF16, kind="ExternalOutput")
        ci = nc.dram_tensor("ci", [BATCH, NSLOT], U32, kind="ExternalOutput")

    with TileContext(nc) as tc:
        with (
            tc.tile_pool(name="consts", bufs=1) as consts,
            tc.tile_pool(name="psum", bufs=4, space="PSUM") as psum,
            tc.tile_pool(name="work", bufs=WBUFS) as work,
            tc.tile_pool(name="cand", bufs=2) as cand,
        ):
            if ROWTILE:
                # Duplicate u and item chunks into both SBUF partition halves:
                # PE tile T0 streams from partitions 0-63, T8 from 64-127.
                u_sb = consts.tile([2 * EMB, BATCH], MDT, tag="u_sb")
                nc.sync.dma_start(u_sb[0:EMB, :], u_t[:])
                nc.sync.dma_start(u_sb[EMB : 2 * EMB, :], u_t[:])
                i_sb = []
                for c in range(NPCH):
                    t = consts.tile([2 * EMB, PCH], MDT, tag=f"i_sb{c}")
                    nc.sync.dma_start(t[0:EMB, :], i_t[:, c * PCH : (c + 1) * PCH])
                    nc.sync.dma_start(
                        t[EMB : 2 * EMB, :], i_t[:, c * PCH : (c + 1) * PCH]
                    )
                    i_sb.append(t)
            else:
                u_sb = consts.tile([EMB, BATCH], MDT, tag="u_sb")
                nc.sync.dma_start(u_sb[:], u_t[:])
                i_sb = []
                for c in range(NPCH):
                    t = consts.tile([EMB, PCH], MDT, tag=f"i_sb{c}")
                    nc.sync.dma_start(t[:], i_t[:, c * PCH : (c + 1) * PCH])
                    i_sb.append(t)

            def body():
                for rt in range(NROWT):
                    s_bf = work.tile([ROWT, NA], BF16, tag="s_bf")
                    s_f32 = (
                        work.tile(
                            [ROWT, DMACH * PCH], F32, name="s_f32", tag="s_f32"
                        )
                        if DMACH
                        else None
                    )
                    f1 = work.tile([ROWT, IPAD // 2], BF16, tag="f1")
                    fl = [
                        work.tile(
                            [ROWT, IPAD >> (l + 1)],
                            BF16,
                            name=f"f{l + 1}",
                            tag=f"f{l + 1}",
                        )
                        for l in range(1, FOLDS)
                    ]
                    if not HOST_TOPK:
                        cv_t = cand.tile([ROWT, NSLOT], BF16, name="cv_t", tag="cv_t")
                        ci_t = cand.tile([ROWT, NSLOT], U32, name="ci_t", tag="ci_t")
                    if ROWTILE:
                        lhs_a = u_sb[0:EMB, rt * ROWT : (rt + 1) * ROWT]
                        lhs_b = u_sb[EMB : 2 * EMB, rt * ROWT : (rt + 1) * ROWT]
                    else:
                        lhs_a = lhs_b = u_sb[:, rt * ROWT : (rt + 1) * ROWT]
                    def emit_mms(ps, c, off):
                        """Two 512-col matmuls for chunk c into ps[:, off:off+1024]."""
                        if ROWTILE:
                            nc.tensor.matmul(
                                ps[:, off : off + 512],
                                lhs_a,
                                i_sb[c][0:EMB, 0:512],
                                start=True,
                                stop=True,
                                tile_position=(0, 0),
                            )
                            nc.tensor.matmul(
                                ps[:, off + 512 : off + 1024],
                                lhs_b,
                                i_sb[c][EMB : 2 * EMB, 512:1024],
                                start=True,
                                stop=True,
                                tile_position=(64, 0),
                            )
                        else:
                            nc.tensor.matmul(
                                ps[:, off : off + 512],
                                lhs_a,
                                i_sb[c][:, 0:512],
                                start=True,
                                stop=True,
                            )
                            nc.tensor.matmul(
                                ps[:, off + 512 : off + 1024],
                                lhs_b,
                                i_sb[c][:, 512:1024],
                                start=True,
                                stop=True,
                            )

                    def emit_consume(ps, c, off):
                        """Act-copy or absorb chunk c living at ps[:, off:off+1024]."""
                        if c < ACT_CHUNKS:
                            nc.scalar.copy(
                                s_bf[:, c * PCH : (c + 1) * PCH],
                                ps[:, off : off + PCH],
                            )
                        else:
                            d = c - ACT_CHUNKS
                            nc.vector.tensor_max(
                                f1[:, d * PCH : (d + 1) * PCH],
                                ps[:, off : off + PCH],
                                s_bf[:, d * PCH : (d + 1) * PCH],
                            )

                    with nc.allow_low_precision(reason="bf16 candidate scores"):
                        if BIGCOPY:
                            for tp in range((NPCH + 1) // 2):
                                c0, c1 = 2 * tp, 2 * tp + 1
                                ps = psum.tile([ROWT, 2 * PCH], F32, tag="ps2", bufs=2)
                                emit_mms(ps, c0, 0)
                                if c1 < NPCH:
                                    emit_mms(ps, c1, PCH)
                                if DIAG == 1:
                                    continue
                                both_act = c1 < ACT_CHUNKS
                                if both_act:
                                    nc.scalar.copy(
                                        s_bf[:, c0 * PCH : (c0 + 2) * PCH], ps[:]
                                    )
                                else:
                                    emit_consume(ps, c0, 0)
                                    if c1 < NPCH:
                                        emit_consume(ps, c1, PCH)
                            if DIAG:
                                continue
                        else:
                            for c in range(NPCH):
                                ps = psum.tile([ROWT, PCH], F32, tag="ps")
                                emit_mms(ps, c, 0)
                                if DIAG == 1:
                                    continue
                                if DIAG == 2:
                                    nc.scalar.copy(
                                        s_bf[:, c * PCH : (c + 1) * PCH], ps[:]
                                    )
                                    continue
                                emit_consume(ps, c, 0)
                        nc.vector.tensor_max(
                            f1[:, ABSORBS * PCH : ABSORBS * PCH + REMW // 2],
                            s_bf[:, ABSORBS * PCH : ABSORBS * PCH + REMW // 2],
                            s_bf[:, ABSORBS * PCH + REMW // 2 : NA],
                        )
                        for m in range(DMACH):
                            lo = ABSORBS * PCH + REMW // 2 + m * (PCH // 2)
                            nc.vector.tensor_max(
                                f1[:, lo : lo + PCH // 2],
                                s_f32[:, m * PCH : m * PCH + PCH // 2],
                                s_f32[:, m * PCH + PCH // 2 : (m + 1) * PCH],
                            )
                        prev = f1
                        for l in range(1, FOLDS):
                            w = IPAD >> (l + 1)
                            if GPFOLD and l == 1:
                                nc.gpsimd.scalar_tensor_tensor(
                                    fl[l - 1][:],
                                    prev[:, 0:w],
                                    1.0,
                                    prev[:, w : 2 * w],
                                    mybir.AluOpType.mult,
                                    mybir.AluOpType.max,
                                )
                            else:
                                nc.vector.tensor_max(
                                    fl[l - 1][:], prev[:, 0:w], prev[:, w : 2 * w]
                                )
                            prev = fl[l - 1]
                        if not HOST_TOPK:
                            nc.vector.max(cv_t[:], prev[:])
                            nc.vector.max_index(ci_t[:], cv_t[:], prev[:])
                    if HOST_TOPK:
                        nc.sync.dma_start(fo[rt * ROWT : (rt + 1) * ROWT, :], prev[:])
                    else:
                        nc.sync.dma_start(cv[rt * ROWT : (rt + 1) * ROWT, :], cv_t[:])
                        nc.sync.dma_start(ci[rt * ROWT : (rt + 1) * ROWT, :], ci_t[:])

            if loop_n == 1:
                body()
            else:
                with tc.For_i(0, loop_n, 1):
                    body()

    nc.compile()
    return nc


def _get_compiled():
    global _compiled
    if _compiled is None:
        _compiled = _build_bass()
    return _compiled


def run_device(u_t, i_t_shards, trace=False, **kwargs):
    from concourse.bass_utils import run_bass_kernel_spmd

    nc = _get_compiled()
    in_maps = [{"u_t": u_t, "i_t": i_t_shards[s]} for s in range(NCORES)]
    return run_bass_kernel_spmd(nc, in_maps, list(range(NCORES)), trace=trace, **kwargs)


def make_device_inputs(all_embed, user_list):
    all_embed = np.asarray(all_embed, dtype=np.float32)
    user_list = np.asarray(user_list)
    u_e = all_embed[user_list.astype(np.int64)]  # [BATCH, EMB]
    i_e = all_embed[N_USERS:]  # [I, E]
    if MM_DT == "bf16":
        import ml_dtypes

        mdt = ml_dtypes.bfloat16
    else:
        mdt = np.float32
    u_t = np.ascontiguousarray(u_e.T).astype(mdt)  # [EMB, BATCH]
    i_t_shards = []
    for s in range(NCORES):
        sh = np.zeros((EMB, IPAD), dtype=mdt)
        sh[:, :ISHARD] = i_e[s * ISHARD : (s + 1) * ISHARD].T.astype(mdt)
        i_t_shards.append(sh)
    return u_e, i_e, u_t, i_t_shards


def _mask_host_scores(s0, pos_pad):
    """Reference masking semantics on the host-scored region: only valid
    positives with local item index < BATCH (== HOST_COLS) are masked."""
    pos_pad = np.asarray(pos_pad)
    item_idx = pos_pad.astype(np.int64) - N_USERS
    valid = (pos_pad >= 0) & (item_idx < HOST_COLS)
    r, c = np.nonzero(valid)
    np.minimum.at(s0, (r, item_idx[r, c]), np.float32(NEG))
    return s0


def _post_host_topk(results, u_e, i_e, pos_pad):
    """Host-side selection over the full per-core group-maxima arrays.

    The device returns every group's bf16 max (no on-device top-8), so
    coverage is decided entirely on host: expand the globally-top T groups
    per row, rescore their members exactly, and stop once the next
    unexpanded group max + margin cannot reach the row's 20th exact value.
    """
    gm = np.stack(
        [np.asarray(results[s]["fo"], dtype=np.float32) for s in range(NCORES)],
        axis=1,
    )  # [BATCH, NCORES, FINAL_W]
    flat = gm.reshape(BATCH, NCORES * FINAL_W)
    order = np.argsort(-flat, axis=1)  # [BATCH, NCORES*FINAL_W]
    rows = np.arange(BATCH)[:, None]

    # Host-exact scores for the maskable region (global item cols [0, 1024)).
    s0 = u_e @ i_e[:HOST_COLS].T
    s0 = _mask_host_scores(s0, pos_pad)
    hp = np.argpartition(-s0, K, axis=1)[:, :K]
    hv = np.take_along_axis(s0, hp, axis=1).astype(np.float32)

    scale = np.abs(flat).max(axis=1)
    margin = REL_EPS * np.maximum(scale, 1.0) + ABS_EPS  # [BATCH]

    # Precompute global member table per (core, pos): [NCORES*FINAL_W, GW]
    gmem = (
        _GTBL[None, :, :] + (np.arange(NCORES, dtype=np.int64) * ISHARD)[:, None, None]
    )
    gmem = np.where(_GTBL[None, :, :] >= 0, gmem, -1)
    gmem = np.where(gmem >= HOST_COLS, gmem, -1).reshape(NCORES * FINAL_W, GW)

    T = 24
    while True:
        sel_groups = order[:, :T]  # [BATCH, T]
        items = gmem[sel_groups].reshape(BATCH, T * GW)
        ok = items >= 0
        safe = np.where(ok, items, 0)
        vals = np.einsum("re,rce->rc", u_e, i_e[safe], optimize=True).astype(np.float32)
        vals[~ok] = -np.inf
        items = np.where(ok, items, -1)

        all_v = np.concatenate([hv, vals], axis=1)
        all_g = np.concatenate([hp.astype(np.int64), items], axis=1)
        v20 = -np.partition(-all_v, K - 1, axis=1)[:, K - 1]

        if T >= NCORES * FINAL_W:
            break
        nxt = flat[rows[:, 0], order[:, T]]  # next unexpanded group max
        if (nxt + margin < v20).all():
            break
        T = min(T * 2, NCORES * FINAL_W)

    # Final top-K with the reference's tie semantics (lower index wins).
    out_idx = np.empty((BATCH, K), dtype=np.int64)
    out_val = np.empty((BATCH, K), dtype=np.float32)
    for r in range(BATCH):
        o = np.lexsort((all_g[r], -all_v[r]))[:K]
        out_idx[r] = all_g[r][o]
        out_val[r] = all_v[r][o]
    return out_idx.astype(np.int32) + N_USERS, out_val


def postprocess(results, u_e, i_e, pos_pad):
    """Decode candidate groups, recompute exact scores, merge, re-select."""
    if HOST_TOPK:
        return _post_host_topk(results, u_e, i_e, pos_pad)
    slot_v = np.empty((NCORES, BATCH, NSLOT), dtype=np.float32)
    slot_p = np.empty((NCORES, BATCH, NSLOT), dtype=np.int64)
    for s in range(NCORES):
        slot_v[s] = np.asarray(results[s]["cv"], dtype=np.float32)
        slot_p[s] = results[s]["ci"].astype(np.int64)

    # Guard against malformed positions (duplicates handled below).
    slot_ok = (slot_p >= 0) & (slot_p < FINAL_W)
    safe_p = np.where(slot_ok, slot_p, 0)

    # Expand groups -> candidate local item columns [NCORES, BATCH, NSLOT*GW]
    members = _GTBL[safe_p]  # [NCORES, BATCH, NSLOT, GW]
    cand_l = members.reshape(NCORES, BATCH, NSLOT * GW)
    core_base = (np.arange(NCORES, dtype=np.int64) * ISHARD)[:, None, None]
    cand_g = np.where(cand_l >= 0, cand_l + core_base, -1)
    cand_ok = (
        (cand_l >= 0) & slot_ok.repeat(GW, axis=2) & (cand_g >= HOST_COLS)
    )

    # [BATCH, NCORES * NSLOT * GW]
    cand_g = cand_g.transpose(1, 0, 2).reshape(BATCH, -1)
    cand_ok = cand_ok.transpose(1, 0, 2).reshape(BATCH, -1)
    safe_g = np.where(cand_ok, cand_g, 0)
    cand_v = np.einsum("re,rce->rc", u_e, i_e[safe_g], optimize=True).astype(np.float32)
    cand_v[~cand_ok] = -np.inf
    cand_g = np.where(cand_ok, cand_g, -1)

    # Host-exact scores for the maskable region (global item cols [0, 1024)).
    s0 = u_e @ i_e[:HOST_COLS].T  # [BATCH, HOST_COLS] float32
    s0 = _mask_host_scores(s0, pos_pad)
    hp = np.argpartition(-s0, K, axis=1)[:, :K]
    hv = np.take_along_axis(s0, hp, axis=1).astype(np.float32)

    all_v = np.concatenate([hv, cand_v], axis=1)
    all_g = np.concatenate([hp.astype(np.int64), cand_g], axis=1)

    order = np.argsort(-all_v, axis=1, kind="stable")[:, : K + 1]
    rows = np.arange(BATCH)[:, None]
    sel_v = all_v[rows, order]
    v20 = sel_v[:, K - 1]

    # Guard: core's 8th group max (bf16) + margin can still reach the row's
    # 20th exact value -> that core may hide candidates; recompute it exactly.
    # Also trigger on duplicate/oob slot positions (tie-loss safety) when the
    # offending value could matter.
    scale = np.maximum(np.abs(sel_v[:, 0]), 1.0)  # [BATCH]
    margin = REL_EPS * scale + ABS_EPS
    trig = slot_v[:, :, NSLOT - 1] + margin[None, :] >= v20[None, :]  # [NCORES,BATCH]
    for s in range(NCORES):
        dup = np.zeros(BATCH, dtype=bool)
        sp = np.sort(slot_p[s], axis=1)
        dup |= (sp[:, 1:] == sp[:, :-1]).any(axis=1)
        dup |= ~slot_ok[s].all(axis=1)
        if dup.any():
            top_margin = slot_v[s, :, 0] + margin >= v20
            trig[s] |= dup & top_margin
    tie = sel_v[:, K - 1] == sel_v[:, K]
    careful = set(np.nonzero(trig.any(axis=0) | tie)[0].tolist())

    out_idx = np.empty((BATCH, K), dtype=np.int64)
    out_val = np.empty((BATCH, K), dtype=np.float32)

    top_g = all_g[rows, order[:, :K]]
    top_v = sel_v[:, :K]
    for r in range(BATCH):
        o = np.lexsort((top_g[r], -top_v[r]))
        out_idx[r] = top_g[r][o]
        out_val[r] = top_v[r][o]

    for r in careful:
        vals = list(all_v[r].astype(np.float64))
        idxs = list(all_g[r])
        recomputed = set()
        while True:
            vv = np.asarray(vals, dtype=np.float64)
            gg = np.asarray(idxs, dtype=np.int64)
            o = np.lexsort((gg, -vv))[:K]
            tg, tv = gg[o], vv[o]
            r20 = tv[-1]
            trig_r = []
            for s in range(NCORES):
                if s in recomputed:
                    continue
                hit = slot_v[s, r, NSLOT - 1] + margin[r] >= r20
                sp = np.sort(slot_p[s, r])
                if (sp[1:] == sp[:-1]).any() or not slot_ok[s, r].all():
                    hit = hit or (slot_v[s, r, 0] + margin[r] >= r20)
                if hit:
                    trig_r.append(s)
            if not trig_r:
                break
            for s in trig_r:
                recomputed.add(s)
                # invalidate the core's original candidates (superseded by
                # the full-shard recompute; avoids duplicate indices)
                base = K + s * NSLOT * GW
                for j in range(base, base + NSLOT * GW):
                    vals[j] = -np.inf
                    idxs[j] = -1
                lo = max(s * ISHARD, HOST_COLS)
                hi = (s + 1) * ISHARD
                if lo >= hi:
                    continue
                sc = (i_e[lo:hi] @ u_e[r]).astype(np.float32)
                vals.extend(sc.tolist())
                idxs.extend(range(lo, hi))
        out_idx[r] = tg
        out_val[r] = tv.astype(np.float32)

    return out_idx.astype(np.int32) + N_USERS, out_val


def kernel(all_embed, pos_pad, user_list, k):
    pos_pad = np.asarray(pos_pad)
    k = int(k)
    assert k == K, f"kernel hardcoded for k={K}, got {k}"
    u_e, i_e, u_t, i_t_shards = make_device_inputs(all_embed, user_list)
    res = run_device(u_t, i_t_shards)
    return postprocess(res.results, u_e, i_e, pos_pad)
